# revision 14
# baseline (speedup 1.0000x reference)
"""AdaptAttention Trainium2 kernel.

8 NeuronCores, data-parallel over batch (16 batches -> 2 per core).
Per (batch, head): computes S'[k,q] (transposed-domain scores) fully on
TensorE PSUM accumulation, exp on ScalarE, P@V + row-sums via a
ones-column, P^T store tiles via PE transpose with 1/Z fused into the
PSUM->SBUF copy.

Returns (out, p_attn_g, p_attn_l) matching the reference module.
"""

import math
import sys

import numpy as np

sys.path.insert(0, "/opt/trn_rl_repo")

import concourse.bacc as bacc
import concourse.bass as bass
import concourse.mybir as mybir
from concourse.bass_utils import run_bass_kernel_spmd
from concourse.masks import make_identity
from concourse.tile import TileContext

AF = mybir.ActivationFunctionType
ALU = mybir.AluOpType
F32 = mybir.dt.float32
BF16 = mybir.dt.bfloat16
I32 = mybir.dt.int32

N = 1024
DK = 64
H = 8
H_GLB = 4
H_LOC = 4
D = 512
B_LOC = 2  # batches per core
N_CORES = 8
P = 128  # partitions
MASKV = 1.0e12


def _derived():
    global NK, NQ, NC2, WN, SPL, NG
    NK = N // P  # k-blocks
    NQ = N // P  # q-chunks
    NC2 = 2 * N // P  # chunks of the rel-pos fold
    WN = N + (NK - 1) * P  # window width
    # <=512-wide column splits of N (psum bank per matmul)
    SPL = [(i, min(512, N - i)) for i in range(0, N, 512)]
    NG = (NQ + 3) // 4  # groups of 4 q-chunks for the out-transpose banks


_derived()


def _affine_fill_diag(nc, ap, val, base, cm, step):
    """Fill ap where (base + cm*partition + step*col) == 0 with val."""
    nc.gpsimd.affine_select(
        out=ap,
        in_=ap,
        compare_op=ALU.not_equal,
        fill=val,
        base=base,
        pattern=[[step, ap.shape[1]]],
        channel_multiplier=cm,
    )


def _load(nc, pool, dram_ap, shape, dtype, tag):
    t = pool.tile(shape, dtype, tag=tag)
    nc.sync.dma_start(out=t, in_=dram_ap)
    return t


def build_nc():
    nc = bacc.Bacc(
        "TRN2",
        target_bir_lowering=False,
        debug=False,
        enable_asserts=False,
        num_devices=N_CORES,
    )

    # ---- I/O ----
    q_d = nc.dram_tensor("q", [B_LOC, H, N, DK], F32, kind="ExternalInput")
    k_d = nc.dram_tensor("k", [B_LOC, H, N, DK], F32, kind="ExternalInput")
    v_d = nc.dram_tensor("v", [B_LOC, H, N, DK], F32, kind="ExternalInput")
    mask_d = nc.dram_tensor("mask", [B_LOC, 1, N, N], I32, kind="ExternalInput")
    users_d = nc.dram_tensor("users", [B_LOC, N, D], F32, kind="ExternalInput")
    rpt_d = nc.dram_tensor("rpt", [2 * N - 1, H_LOC * DK], F32, kind="ExternalInput")
    upw_d = nc.dram_tensor("upw", [D, H_LOC * DK], F32, kind="ExternalInput")
    upb_d = nc.dram_tensor("upb", [H_LOC * DK], F32, kind="ExternalInput")
    mlw_d = nc.dram_tensor("mlw", [H_LOC, DK], F32, kind="ExternalInput")
    mlb_d = nc.dram_tensor("mlb", [H_LOC], F32, kind="ExternalInput")

    out_d = nc.dram_tensor("out", [B_LOC, H, N, DK], F32, kind="ExternalOutput")
    pg_d = nc.dram_tensor("pg", [B_LOC, H_GLB, N, N], F32, kind="ExternalOutput")
    pl_d = nc.dram_tensor("pl", [B_LOC, H_LOC, N, N], F32, kind="ExternalOutput")

    scale = 1.0 / math.sqrt(DK)

    with TileContext(nc) as tc:
        with (
            tc.tile_pool(name="consts", bufs=1) as consts,
            tc.tile_pool(name="wwin", bufs=1) as wwin,
            tc.tile_pool(name="dram", bufs=1, space="DRAM") as dpool,
            tc.tile_pool(name="ps_s", bufs=2, space="PSUM") as ps_s,
            tc.tile_pool(name="ps_acc", bufs=1, space="PSUM") as ps_acc,
            tc.tile_pool(name="ps_big", bufs=1, space="PSUM") as ps_big,
        ):
            # ================= prologue (once) =================
            ident = consts.tile([P, P], F32)
            make_identity(nc, ident)
            jmat_f = consts.tile([P, P], F32)
            nc.gpsimd.memset(jmat_f, 0.0)
            _affine_fill_diag(nc, jmat_f, 1.0, base=-(P - 1), cm=1, step=1)
            ident_b = consts.tile([P, P], BF16)
            nc.vector.tensor_copy(ident_b, ident)
            jmat_b = consts.tile([P, P], BF16)
            nc.vector.tensor_copy(jmat_b, jmat_f)

            # mlp_w broadcast [128, 256], flat row [1, 256], per-head cols
            mw_b = consts.tile([P, H_LOC * DK], F32)
            nc.sync.dma_start(
                out=mw_b,
                in_=bass.AP(tensor=mlw_d, offset=0, ap=[[0, P], [1, H_LOC * DK]]),
            )
            ones_row = consts.tile([1, N], F32)
            nc.vector.memset(ones_row, 1.0)
            mw_cols = []
            for h in range(H_LOC):
                t = consts.tile([DK, 1], F32, tag=f"mwc{h}")
                nc.sync.dma_start(
                    out=t,
                    in_=bass.AP(tensor=mlw_d, offset=h * DK, ap=[[1, DK], [0, 1]]),
                )
                mw_cols.append(t)

            # W_U [512, 4] chunks + c_u' [1, 4]; rel-pos fold -> W windows
            wu_chunks = [consts.tile([P, H_LOC], F32, tag=f"wu{dc}", name=f"wu{dc}") for dc in range(D // P)]
            cu_sb = consts.tile([1, H_LOC], F32)
            w_tiles = [wwin.tile([P, WN], BF16, tag=f"w{h}", name=f"wwin{h}") for h in range(H_LOC)]
            t_rev_dram = dpool.tile([H_LOC, 2 * N], F32)

            with tc.tile_pool(name="prol", bufs=2) as prol:
                mw_row = prol.tile([1, H_LOC * DK], F32, tag="mwrow")
                nc.sync.dma_start(
                    out=mw_row,
                    in_=bass.AP(
                        tensor=mlw_d, offset=0, ap=[[0, 1], [1, H_LOC * DK]]
                    ),
                )
                for dc in range(D // P):
                    upw_t = prol.tile([P, H_LOC * DK], F32, tag="upw")
                    nc.sync.dma_start(
                        out=upw_t, in_=upw_d.ap()[dc * P : (dc + 1) * P, :]
                    )
                    tmp = prol.tile([P, H_LOC * DK], F32, tag="wutmp")
                    nc.vector.tensor_tensor(out=tmp, in0=upw_t, in1=mw_b, op=ALU.mult)
                    nc.vector.tensor_reduce(
                        out=wu_chunks[dc],
                        in_=tmp.rearrange("p (h d) -> p h d", d=DK),
                        axis=mybir.AxisListType.X,
                        op=ALU.add,
                    )

                upb_t = prol.tile([1, H_LOC * DK], F32, tag="upb")
                nc.sync.dma_start(
                    out=upb_t,
                    in_=bass.AP(tensor=upb_d, offset=0, ap=[[0, 1], [1, H_LOC * DK]]),
                )
                cu_tmp = prol.tile([1, H_LOC * DK], F32, tag="cutmp")
                nc.vector.tensor_tensor(out=cu_tmp, in0=upb_t, in1=mw_row, op=ALU.mult)
                nc.vector.tensor_reduce(
                    out=cu_sb,
                    in_=cu_tmp.rearrange("p (h d) -> p h d", d=DK),
                    axis=mybir.AxisListType.X,
                    op=ALU.add,
                )
                mlb_t = prol.tile([1, H_LOC], F32, tag="mlb")
                nc.sync.dma_start(
                    out=mlb_t,
                    in_=bass.AP(tensor=mlb_d, offset=0, ap=[[0, 1], [1, H_LOC]]),
                )
                nc.vector.tensor_tensor(out=cu_sb, in0=cu_sb, in1=mlb_t, op=ALU.add)

                # T4[p, c, h] = t~[128c + p], t~[i] = t[i-1] (t~[0] junk)
                t4 = prol.tile([P, NC2, H_LOC], F32, tag="t4")
                for c in range(NC2):
                    rc = prol.tile([P, H_LOC * DK], F32, tag="rc")
                    lo = c * P - 1
                    if lo < 0:
                        nc.sync.dma_start(out=rc[1:P, :], in_=rpt_d.ap()[0 : P - 1, :])
                        nc.vector.memset(rc[0:1, :], 0.0)
                    else:
                        nc.sync.dma_start(out=rc, in_=rpt_d.ap()[lo : lo + P, :])
                    tmp2 = prol.tile([P, H_LOC * DK], F32, tag="rcm")
                    nc.vector.tensor_tensor(out=tmp2, in0=rc, in1=mw_b, op=ALU.mult)
                    nc.vector.tensor_reduce(
                        out=t4[:, c],
                        in_=tmp2.rearrange("p (h d) -> p h d", d=DK),
                        axis=mybir.AxisListType.X,
                        op=ALU.add,
                    )

                # t_rev[h, i] = t~[2047 - i]: J-flip partitions, reverse cols
                for h in range(H_LOC):
                    ps_jt = ps_big.tile([P, NC2], F32, tag="big")
                    t4h = prol.tile([P, NC2], BF16, tag="t4h")
                    nc.vector.tensor_copy(out=t4h, in_=t4[:, :, h])
                    nc.tensor.matmul(ps_jt, jmat_b, t4h, start=True, stop=True)
                    tr_sb = prol.tile([P, NC2], F32, tag="trsb")
                    for c in range(NC2):
                        nc.vector.tensor_copy(
                            out=tr_sb[:, NC2 - 1 - c : NC2 - c], in_=ps_jt[:, c : c + 1]
                        )
                    nc.sync.dma_start(
                        out=bass.AP(
                            tensor=t_rev_dram.tensor,
                            offset=t_rev_dram.offset + h * 2 * N,
                            ap=[[1, P], [P, NC2]],
                        ),
                        in_=tr_sb,
                    )
                    # W_h[p, c] = t_rev[p + c] (overlapping windows), bf16
                    wtmp = prol.tile([P, WN], F32, tag="wtmp")
                    nc.sync.dma_start(
                        out=wtmp,
                        in_=bass.AP(
                            tensor=t_rev_dram.tensor,
                            offset=t_rev_dram.offset + h * 2 * N,
                            ap=[[1, P], [1, WN]],
                        ),
                    )
                    nc.vector.tensor_copy(w_tiles[h], wtmp)

            # ================= main =================
            with (
                tc.tile_pool(name="maskT", bufs=1) as maskp,
                tc.tile_pool(name="mload", bufs=2) as mload,
                tc.tile_pool(name="batch", bufs=1) as batchp,
                tc.tile_pool(name="qk", bufs=2) as qkp,
                tc.tile_pool(name="vpool", bufs=2) as vpoolp,
                tc.tile_pool(name="hsing", bufs=1) as hsing,
                tc.tile_pool(name="pp", bufs=8) as ppool,
                tc.tile_pool(name="pstore", bufs=2) as pstorep,
                tc.tile_pool(name="small", bufs=2) as smallp,
            ):
                for b in range(B_LOC):
                    # ---- maskT tiles [128k, 1024q] bf16 (0 valid / -1e12)
                    maskT = [maskp.tile([P, N], BF16, tag=f"mt{kc}", name=f"mt{kc}") for kc in range(NK)]
                    for qc in range(NQ):
                        mi = mload.tile([P, N], I32, tag="mi")
                        nc.sync.dma_start(
                            out=mi, in_=mask_d.ap()[b, 0, qc * P : (qc + 1) * P, :]
                        )
                        mf = mload.tile([P, N], F32, tag="mf")
                        nc.scalar.activation(
                            out=mf, in_=mi, func=AF.Copy, bias=-MASKV, scale=MASKV
                        )
                        ps_mt = ps_big.tile([P, N], F32, tag="big")
                        for kc in range(NK):
                            nc.tensor.transpose(
                                ps_mt[:, kc * P : (kc + 1) * P],
                                mf[:, kc * P : (kc + 1) * P],
                                ident,
                            )
                        for kc in range(NK):
                            nc.any.tensor_copy(
                                maskT[kc][:, qc * P : (qc + 1) * P],
                                ps_mt[:, kc * P : (kc + 1) * P],
                            )

                    # ---- users^T (persists for this batch, feeds per-head ru)
                    ut_tiles = []
                    for dc in range(D // P):
                        ps_ut = ps_big.tile([P, N], F32, tag="big")
                        for qc in range(NQ):
                            u_in = _load(
                                nc,
                                batchp,
                                users_d.ap()[b][
                                    qc * P : (qc + 1) * P, dc * P : (dc + 1) * P
                                ],
                                [P, P],
                                F32,
                                tag="uld",
                            )
                            nc.tensor.transpose(
                                ps_ut[:, qc * P : (qc + 1) * P], u_in, ident
                            )
                        ut = batchp.tile([P, N], F32, tag=f"ut{dc}")
                        nc.any.tensor_copy(ut, ps_ut)
                        ut_tiles.append(ut)

                    # ================= per head =================
                    for h in range(H):
                        loc = h >= H_GLB
                        hl = h - H_GLB
                        kdim = 66 if loc else 64

                        # Q^T (scaled) -> qp rows 0..63
                        qp = qkp.tile([66, N], F32, tag="qp")
                        ps_qt = ps_big.tile([P, N], F32, tag="big")
                        for qc in range(NQ):
                            q_in = _load(
                                nc, qkp,
                                q_d.ap()[b, h][qc * P : (qc + 1) * P, :],
                                [P, DK], F32, tag="qld",
                            )
                            nc.tensor.transpose(
                                ps_qt[0:DK, qc * P : (qc + 1) * P], q_in, ident
                            )
                        nc.any.tensor_scalar_mul(qp[0:DK, :], ps_qt[0:DK, :], scale)

                        # K^T -> kp rows 0..63
                        kp = qkp.tile([66, N], F32, tag="kp")
                        ps_kt = ps_big.tile([P, N], F32, tag="big")
                        for kc in range(NK):
                            k_in = _load(
                                nc, qkp,
                                k_d.ap()[b, h][kc * P : (kc + 1) * P, :],
                                [P, DK], F32, tag="kld",
                            )
                            nc.tensor.transpose(
                                ps_kt[0:DK, kc * P : (kc + 1) * P], k_in, ident
                            )
                        nc.any.tensor_copy(kp[0:DK, :], ps_kt[0:DK, :])

                        # V tiles with ones column
                        vp_tiles = []
                        for kc in range(NK):
                            vp = vpoolp.tile([P, DK + 1], F32, tag=f"vp{kc}")
                            nc.sync.dma_start(
                                out=vp[:, 0:DK],
                                in_=v_d.ap()[b, h][kc * P : (kc + 1) * P, :],
                            )
                            nc.vector.memset(vp[:, DK : DK + 1], 1.0)
                            vp_tiles.append(vp)

                        if loc:
                            # V^T, rv = mlp_w[h].V^T; perq = rv + ru + cu
                            ps_vt = ps_big.tile([P, N], F32, tag="big")
                            for kc in range(NK):
                                nc.tensor.transpose(
                                    ps_vt[0:DK, kc * P : (kc + 1) * P],
                                    vp_tiles[kc][:, 0:DK],
                                    ident,
                                )
                            vt = hsing.tile([DK, N], F32, tag="vt")
                            nc.any.tensor_copy(vt, ps_vt[0:DK, :])
                            ps_rv = ps_big.tile([1, N], F32, tag="big")
                            for (s0, sw) in SPL:
                                nc.tensor.matmul(
                                    ps_rv[:, s0 : s0 + sw],
                                    mw_cols[hl],
                                    vt[:, s0 : s0 + sw],
                                    start=True,
                                    stop=True,
                                )
                            rv = hsing.tile([1, N], F32, tag="rv")
                            nc.any.tensor_copy(rv, ps_rv)
                            # ru in psum: sum_dc WU[dc]^T . ut[dc]
                            ps_ru = ps_big.tile([1, N], F32, tag="big")
                            for dc in range(D // P):
                                for (s0, sw) in SPL:
                                    nc.tensor.matmul(
                                        ps_ru[:, s0 : s0 + sw],
                                        wu_chunks[dc][:, hl : hl + 1],
                                        ut_tiles[dc][:, s0 : s0 + sw],
                                        start=(dc == 0),
                                        stop=(dc == D // P - 1),
                                    )
                            perq = hsing.tile([1, N], F32, tag="perq")
                            nc.vector.tensor_scalar(
                                out=perq,
                                in0=ps_ru,
                                scalar1=cu_sb[0:1, hl : hl + 1],
                                scalar2=None,
                                op0=ALU.add,
                            )
                            nc.vector.tensor_tensor(
                                out=perq, in0=perq, in1=rv, op=ALU.add
                            )
                            nc.sync.dma_start(out=qp[64:65, :], in_=perq)
                            nc.sync.dma_start(out=qp[65:66, :], in_=ones_row)
                            nc.vector.memset(kp[64:65, :], 1.0)
                            nc.sync.dma_start(out=kp[65:66, :], in_=rv)

                        # ---- main k-block loop
                        ps_ot = ps_acc.tile([DK + 1, N], F32, tag="ot")
                        pp_tiles = []
                        for j in range(NK):
                            ps = ps_s.tile([P, N], F32, tag="s")
                            for (s0, sw) in SPL:
                                sl = slice(s0, s0 + sw)
                                nc.tensor.matmul(
                                    ps[:, sl],
                                    kp[0:kdim, j * P : (j + 1) * P],
                                    qp[0:kdim, sl],
                                    start=True,
                                    stop=False,
                                )
                                if loc:
                                    off = (NK - 1 - j) * P + s0
                                    nc.tensor.matmul(
                                        ps[:, sl],
                                        jmat_b,
                                        w_tiles[hl][:, off : off + sw],
                                        start=False,
                                        stop=False,
                                    )
                                nc.tensor.matmul(
                                    ps[:, sl],
                                    ident_b,
                                    maskT[j][:, sl],
                                    start=False,
                                    stop=True,
                                )
                            pp = ppool.tile([P, N], F32, tag="pp")
                            nc.scalar.activation(out=pp, in_=ps, func=AF.Exp)
                            pp_tiles.append(pp)
                            for (s0, sw) in SPL:
                                sl = slice(s0, s0 + sw)
                                nc.tensor.matmul(
                                    ps_ot[:, sl],
                                    vp_tiles[j],
                                    pp[:, sl],
                                    start=(j == 0),
                                    stop=(j == NK - 1),
                                )

                        # ---- outT -> SBUF; transpose incl. Z row; 1/Z
                        ot_sb = smallp.tile([DK + 1, N], F32, tag="otsb")
                        nc.any.tensor_copy(ot_sb, ps_ot)
                        ps_o = ps_big.tile([P, NG, 512], F32, tag="big")
                        for qc in range(NQ):
                            g, i = divmod(qc, 4)
                            nc.tensor.transpose(
                                ps_o[:, g, i * 65 : i * 65 + 65],
                                ot_sb[0 : DK + 1, qc * P : (qc + 1) * P],
                                ident[0 : DK + 1, 0 : DK + 1],
                            )
                        rz = smallp.tile([P, NQ], F32, tag="rz")
                        for g in range(NG):
                            ng = min(4, NQ - g * 4)
                            nc.vector.reciprocal(
                                out=rz[:, g * 4 : g * 4 + ng],
                                in_=ps_o[:, g, 64 : ng * 65 : 65],
                            )
                        o_sb = smallp.tile([P, NQ * DK], F32, tag="osb")
                        for qc in range(NQ):
                            g, i = divmod(qc, 4)
                            nc.any.tensor_scalar(
                                out=o_sb[:, qc * DK : (qc + 1) * DK],
                                in0=ps_o[:, g, i * 65 : i * 65 + DK],
                                scalar1=rz[:, qc : qc + 1],
                                scalar2=None,
                                op0=ALU.mult,
                            )
                        nc.sync.dma_start(
                            out=out_d.ap()[b, h].rearrange("(qc p) d -> p qc d", p=P),
                            in_=o_sb.rearrange("p (qc d) -> p qc d", d=DK),
                        )

                        # ---- P store: transpose chunks, scale by 1/Z
                        p_out = pl_d.ap()[b, hl] if loc else pg_d.ap()[b, h]
                        for qc in range(NQ):
                            ps_pt = ps_big.tile([P, N], F32, tag="big")
                            for kc in range(NK):
                                nc.tensor.transpose(
                                    ps_pt[:, kc * P : (kc + 1) * P],
                                    pp_tiles[kc][:, qc * P : (qc + 1) * P],
                                    ident,
                                )
                            pst = pstorep.tile([P, N], F32, tag="pst")
                            nc.any.tensor_scalar(
                                out=pst,
                                in0=ps_pt,
                                scalar1=rz[:, qc : qc + 1],
                                scalar2=None,
                                op0=ALU.mult,
                            )
                            nc.sync.dma_start(
                                out=p_out[qc * P : (qc + 1) * P, :], in_=pst
                            )

    nc.compile()
    return nc


_NC_CACHE = None


def _get_nc():
    global _NC_CACHE
    if _NC_CACHE is None:
        _NC_CACHE = build_nc()
    return _NC_CACHE


def _shard_inputs(inputs):
    q = np.ascontiguousarray(np.asarray(inputs["query"], dtype=np.float32))
    k = np.ascontiguousarray(np.asarray(inputs["key"], dtype=np.float32))
    v = np.ascontiguousarray(np.asarray(inputs["value"], dtype=np.float32))
    mask = np.ascontiguousarray(np.asarray(inputs["mask"], dtype=np.int32))
    users = np.ascontiguousarray(np.asarray(inputs["users"], dtype=np.float32))
    rpt = np.ascontiguousarray(np.asarray(inputs["rel_pos_table"], dtype=np.float32))
    upw = np.ascontiguousarray(np.asarray(inputs["user_proj_w"], dtype=np.float32))
    upb = np.ascontiguousarray(np.asarray(inputs["user_proj_b"], dtype=np.float32))
    mlw = np.ascontiguousarray(np.asarray(inputs["mlp_w"], dtype=np.float32))
    mlb = np.ascontiguousarray(np.asarray(inputs["mlp_b"], dtype=np.float32))
    in_maps = []
    for i in range(N_CORES):
        sl = slice(i * B_LOC, (i + 1) * B_LOC)
        in_maps.append(
            {
                "q": q[sl], "k": k[sl], "v": v[sl], "mask": mask[sl],
                "users": users[sl], "rpt": rpt, "upw": upw, "upb": upb,
                "mlw": mlw, "mlb": mlb,
            }
        )
    return in_maps


def run_sharded(inputs, trace=False):
    nc = _get_nc()
    in_maps = _shard_inputs(inputs)
    res = run_bass_kernel_spmd(
        nc, in_maps, core_ids=list(range(N_CORES)), trace=trace
    )
    outs = res.results
    out = np.concatenate([r["out"] for r in outs], axis=0)
    pg = np.concatenate([r["pg"] for r in outs], axis=0)
    pl = np.concatenate([r["pl"] for r in outs], axis=0)
    return (out, pg, pl), res


def kernel(**inputs):
    (out, pg, pl), _ = run_sharded(inputs, trace=False)
    return (out, pg, pl)


if __name__ == "__main__":
    build_nc()
    print("build ok")


# revision 15
# speedup vs baseline: 1.2360x; 1.2360x over previous
"""AdaptAttention Trainium2 kernel.

8 NeuronCores, data-parallel over batch (16 batches -> 2 per core).
Per (batch, head): computes S'[k,q] (transposed-domain scores) fully on
TensorE PSUM accumulation, exp on ScalarE, P@V + row-sums via a
ones-column, P^T store tiles via PE transpose with 1/Z fused into the
PSUM->SBUF copy.

Returns (out, p_attn_g, p_attn_l) matching the reference module.
"""

import math
import sys

import numpy as np

sys.path.insert(0, "/opt/trn_rl_repo")

import concourse.bacc as bacc
import concourse.bass as bass
import concourse.mybir as mybir
from concourse.bass_utils import run_bass_kernel_spmd
from concourse.masks import make_identity
from concourse.tile import TileContext

AF = mybir.ActivationFunctionType
ALU = mybir.AluOpType
F32 = mybir.dt.float32
BF16 = mybir.dt.bfloat16
I32 = mybir.dt.int32

N = 1024
DK = 64
H = 8
H_GLB = 4
H_LOC = 4
D = 512
B_LOC = 2  # batches per core
N_CORES = 8
P = 128  # partitions
MASKV = 1.0e12


def _derived():
    global NK, NQ, NC2, WN, SPL, NG
    NK = N // P  # k-blocks
    NQ = N // P  # q-chunks
    NC2 = 2 * N // P  # chunks of the rel-pos fold
    WN = N + (NK - 1) * P  # window width
    # <=512-wide column splits of N (psum bank per matmul)
    SPL = [(i, min(512, N - i)) for i in range(0, N, 512)]
    NG = (NQ + 3) // 4  # groups of 4 q-chunks for the out-transpose banks


_derived()


def _affine_fill_diag(nc, ap, val, base, cm, step):
    """Fill ap where (base + cm*partition + step*col) == 0 with val."""
    nc.gpsimd.affine_select(
        out=ap,
        in_=ap,
        compare_op=ALU.not_equal,
        fill=val,
        base=base,
        pattern=[[step, ap.shape[1]]],
        channel_multiplier=cm,
    )


def _load(nc, pool, dram_ap, shape, dtype, tag):
    t = pool.tile(shape, dtype, tag=tag)
    nc.sync.dma_start(out=t, in_=dram_ap)
    return t


def build_nc():
    nc = bacc.Bacc(
        "TRN2",
        target_bir_lowering=False,
        debug=False,
        enable_asserts=False,
        num_devices=N_CORES,
    )

    # ---- I/O ----
    q_d = nc.dram_tensor("q", [B_LOC, H, N, DK], F32, kind="ExternalInput")
    k_d = nc.dram_tensor("k", [B_LOC, H, N, DK], F32, kind="ExternalInput")
    v_d = nc.dram_tensor("v", [B_LOC, H, N, DK], F32, kind="ExternalInput")
    mask_d = nc.dram_tensor("mask", [B_LOC, 1, N, N], I32, kind="ExternalInput")
    users_d = nc.dram_tensor("users", [B_LOC, N, D], F32, kind="ExternalInput")
    rpt_d = nc.dram_tensor("rpt", [2 * N - 1, H_LOC * DK], F32, kind="ExternalInput")
    upw_d = nc.dram_tensor("upw", [D, H_LOC * DK], F32, kind="ExternalInput")
    upb_d = nc.dram_tensor("upb", [H_LOC * DK], F32, kind="ExternalInput")
    mlw_d = nc.dram_tensor("mlw", [H_LOC, DK], F32, kind="ExternalInput")
    mlb_d = nc.dram_tensor("mlb", [H_LOC], F32, kind="ExternalInput")

    out_d = nc.dram_tensor("out", [B_LOC, H, N, DK], F32, kind="ExternalOutput")
    pg_d = nc.dram_tensor("pg", [B_LOC, H_GLB, N, N], F32, kind="ExternalOutput")
    pl_d = nc.dram_tensor("pl", [B_LOC, H_LOC, N, N], F32, kind="ExternalOutput")

    scale = 1.0 / math.sqrt(DK)

    with TileContext(nc) as tc:
        with (
            tc.tile_pool(name="consts", bufs=1) as consts,
            tc.tile_pool(name="wwin", bufs=1) as wwin,
            tc.tile_pool(name="dram", bufs=1, space="DRAM") as dpool,
            tc.tile_pool(name="ps_s", bufs=2, space="PSUM") as ps_s,
            tc.tile_pool(name="ps_acc", bufs=1, space="PSUM") as ps_acc,
            tc.tile_pool(name="ps_big", bufs=2, space="PSUM") as ps_big,
        ):
            # ================= prologue (once) =================
            ident = consts.tile([P, P], F32)
            make_identity(nc, ident)
            jmat_f = consts.tile([P, P], F32)
            nc.gpsimd.memset(jmat_f, 0.0)
            _affine_fill_diag(nc, jmat_f, 1.0, base=-(P - 1), cm=1, step=1)
            ident_b = consts.tile([P, P], BF16)
            nc.vector.tensor_copy(ident_b, ident)
            jmat_b = consts.tile([P, P], BF16)
            nc.vector.tensor_copy(jmat_b, jmat_f)

            # mlp_w broadcast [128, 256], flat row [1, 256], per-head cols
            mw_b = consts.tile([P, H_LOC * DK], F32)
            nc.sync.dma_start(
                out=mw_b,
                in_=bass.AP(tensor=mlw_d, offset=0, ap=[[0, P], [1, H_LOC * DK]]),
            )
            ones_row = consts.tile([1, N], F32)
            nc.vector.memset(ones_row, 1.0)
            mw_cols = []
            for h in range(H_LOC):
                t = consts.tile([DK, 1], F32, tag=f"mwc{h}")
                nc.sync.dma_start(
                    out=t,
                    in_=bass.AP(tensor=mlw_d, offset=h * DK, ap=[[1, DK], [0, 1]]),
                )
                mw_cols.append(t)

            # W_U [512, 4] chunks + c_u' [1, 4]; rel-pos fold -> W windows
            wu_chunks = [consts.tile([P, H_LOC], F32, tag=f"wu{dc}", name=f"wu{dc}") for dc in range(D // P)]
            cu_sb = consts.tile([1, H_LOC], F32)
            w_tiles = [wwin.tile([P, WN], BF16, tag=f"w{h}", name=f"wwin{h}") for h in range(H_LOC)]
            t_rev_dram = dpool.tile([H_LOC, 2 * N], F32)

            with tc.tile_pool(name="prol", bufs=2) as prol:
                mw_row = prol.tile([1, H_LOC * DK], F32, tag="mwrow")
                nc.sync.dma_start(
                    out=mw_row,
                    in_=bass.AP(
                        tensor=mlw_d, offset=0, ap=[[0, 1], [1, H_LOC * DK]]
                    ),
                )
                for dc in range(D // P):
                    upw_t = prol.tile([P, H_LOC * DK], F32, tag="upw")
                    nc.sync.dma_start(
                        out=upw_t, in_=upw_d.ap()[dc * P : (dc + 1) * P, :]
                    )
                    tmp = prol.tile([P, H_LOC * DK], F32, tag="wutmp")
                    nc.vector.tensor_tensor(out=tmp, in0=upw_t, in1=mw_b, op=ALU.mult)
                    nc.vector.tensor_reduce(
                        out=wu_chunks[dc],
                        in_=tmp.rearrange("p (h d) -> p h d", d=DK),
                        axis=mybir.AxisListType.X,
                        op=ALU.add,
                    )

                upb_t = prol.tile([1, H_LOC * DK], F32, tag="upb")
                nc.sync.dma_start(
                    out=upb_t,
                    in_=bass.AP(tensor=upb_d, offset=0, ap=[[0, 1], [1, H_LOC * DK]]),
                )
                cu_tmp = prol.tile([1, H_LOC * DK], F32, tag="cutmp")
                nc.vector.tensor_tensor(out=cu_tmp, in0=upb_t, in1=mw_row, op=ALU.mult)
                nc.vector.tensor_reduce(
                    out=cu_sb,
                    in_=cu_tmp.rearrange("p (h d) -> p h d", d=DK),
                    axis=mybir.AxisListType.X,
                    op=ALU.add,
                )
                mlb_t = prol.tile([1, H_LOC], F32, tag="mlb")
                nc.sync.dma_start(
                    out=mlb_t,
                    in_=bass.AP(tensor=mlb_d, offset=0, ap=[[0, 1], [1, H_LOC]]),
                )
                nc.vector.tensor_tensor(out=cu_sb, in0=cu_sb, in1=mlb_t, op=ALU.add)

                # T4[p, c, h] = t~[128c + p], t~[i] = t[i-1] (t~[0] junk)
                t4 = prol.tile([P, NC2, H_LOC], F32, tag="t4")
                for c in range(NC2):
                    rc = prol.tile([P, H_LOC * DK], F32, tag="rc")
                    lo = c * P - 1
                    if lo < 0:
                        nc.sync.dma_start(out=rc[1:P, :], in_=rpt_d.ap()[0 : P - 1, :])
                        nc.vector.memset(rc[0:1, :], 0.0)
                    else:
                        nc.sync.dma_start(out=rc, in_=rpt_d.ap()[lo : lo + P, :])
                    tmp2 = prol.tile([P, H_LOC * DK], F32, tag="rcm")
                    nc.vector.tensor_tensor(out=tmp2, in0=rc, in1=mw_b, op=ALU.mult)
                    nc.vector.tensor_reduce(
                        out=t4[:, c],
                        in_=tmp2.rearrange("p (h d) -> p h d", d=DK),
                        axis=mybir.AxisListType.X,
                        op=ALU.add,
                    )

                # t_rev[h, i] = t~[2047 - i]: J-flip partitions, reverse cols
                for h in range(H_LOC):
                    ps_jt = ps_big.tile([P, NC2], F32, tag="big")
                    t4h = prol.tile([P, NC2], BF16, tag="t4h")
                    nc.vector.tensor_copy(out=t4h, in_=t4[:, :, h])
                    nc.tensor.matmul(ps_jt, jmat_b, t4h, start=True, stop=True)
                    tr_sb = prol.tile([P, NC2], F32, tag="trsb")
                    for c in range(NC2):
                        nc.vector.tensor_copy(
                            out=tr_sb[:, NC2 - 1 - c : NC2 - c], in_=ps_jt[:, c : c + 1]
                        )
                    nc.sync.dma_start(
                        out=bass.AP(
                            tensor=t_rev_dram.tensor,
                            offset=t_rev_dram.offset + h * 2 * N,
                            ap=[[1, P], [P, NC2]],
                        ),
                        in_=tr_sb,
                    )
                    # W_h[p, c] = t_rev[p + c] (overlapping windows), bf16
                    wtmp = prol.tile([P, WN], F32, tag="wtmp")
                    nc.sync.dma_start(
                        out=wtmp,
                        in_=bass.AP(
                            tensor=t_rev_dram.tensor,
                            offset=t_rev_dram.offset + h * 2 * N,
                            ap=[[1, P], [1, WN]],
                        ),
                    )
                    nc.vector.tensor_copy(w_tiles[h], wtmp)

            # ================= main =================
            with (
                tc.tile_pool(name="maskT", bufs=1) as maskp,
                tc.tile_pool(name="mload", bufs=2) as mload,
                tc.tile_pool(name="batch", bufs=1) as batchp,
                tc.tile_pool(name="qk", bufs=2) as qkp,
                tc.tile_pool(name="vpool", bufs=2) as vpoolp,
                tc.tile_pool(name="hsing", bufs=1) as hsing,
                tc.tile_pool(name="pp", bufs=10) as ppool,
                tc.tile_pool(name="pstore", bufs=2) as pstorep,
                tc.tile_pool(name="small", bufs=2) as smallp,
            ):
                for b in range(B_LOC):
                    # ---- maskT tiles [128k, 1024q] bf16 (0 valid / -1e12)
                    maskT = [maskp.tile([P, N], BF16, tag=f"mt{kc}", name=f"mt{kc}") for kc in range(NK)]
                    for qc in range(NQ):
                        mi = mload.tile([P, N], I32, tag="mi")
                        nc.sync.dma_start(
                            out=mi, in_=mask_d.ap()[b, 0, qc * P : (qc + 1) * P, :]
                        )
                        mf = mload.tile([P, N], F32, tag="mf")
                        nc.scalar.activation(
                            out=mf, in_=mi, func=AF.Copy, bias=-MASKV, scale=MASKV
                        )
                        ps_mt = ps_big.tile([P, N], F32, tag="big")
                        for kc in range(NK):
                            nc.tensor.transpose(
                                ps_mt[:, kc * P : (kc + 1) * P],
                                mf[:, kc * P : (kc + 1) * P],
                                ident,
                            )
                        for kc in range(NK):
                            nc.vector.tensor_copy(
                                maskT[kc][:, qc * P : (qc + 1) * P],
                                ps_mt[:, kc * P : (kc + 1) * P],
                            )

                    # ---- users^T (persists for this batch, feeds per-head ru)
                    ut_tiles = []
                    for dc in range(D // P):
                        ps_ut = ps_big.tile([P, N], F32, tag="big")
                        for qc in range(NQ):
                            u_in = _load(
                                nc,
                                batchp,
                                users_d.ap()[b][
                                    qc * P : (qc + 1) * P, dc * P : (dc + 1) * P
                                ],
                                [P, P],
                                F32,
                                tag="uld",
                            )
                            nc.tensor.transpose(
                                ps_ut[:, qc * P : (qc + 1) * P], u_in, ident
                            )
                        ut = batchp.tile([P, N], F32, tag=f"ut{dc}")
                        nc.vector.tensor_copy(ut, ps_ut)
                        ut_tiles.append(ut)

                    # ================= per head =================
                    for h in range(H):
                        loc = h >= H_GLB
                        hl = h - H_GLB
                        kdim = 66 if loc else 64

                        # Q^T (scaled) -> qp rows 0..63
                        qp = qkp.tile([66, N], F32, tag="qp")
                        ps_qt = ps_big.tile([P, N], F32, tag="big")
                        for qc in range(NQ):
                            q_in = _load(
                                nc, qkp,
                                q_d.ap()[b, h][qc * P : (qc + 1) * P, :],
                                [P, DK], F32, tag="qld",
                            )
                            nc.tensor.transpose(
                                ps_qt[0:DK, qc * P : (qc + 1) * P], q_in, ident
                            )
                        nc.vector.tensor_scalar_mul(qp[0:DK, :], ps_qt[0:DK, :], scale)

                        # K^T -> kp rows 0..63
                        kp = qkp.tile([66, N], F32, tag="kp")
                        ps_kt = ps_big.tile([P, N], F32, tag="big")
                        for kc in range(NK):
                            k_in = _load(
                                nc, qkp,
                                k_d.ap()[b, h][kc * P : (kc + 1) * P, :],
                                [P, DK], F32, tag="kld",
                            )
                            nc.tensor.transpose(
                                ps_kt[0:DK, kc * P : (kc + 1) * P], k_in, ident
                            )
                        nc.vector.tensor_copy(kp[0:DK, :], ps_kt[0:DK, :])

                        # V tiles with ones column
                        vp_tiles = []
                        for kc in range(NK):
                            vp = vpoolp.tile([P, DK + 1], F32, tag=f"vp{kc}")
                            nc.sync.dma_start(
                                out=vp[:, 0:DK],
                                in_=v_d.ap()[b, h][kc * P : (kc + 1) * P, :],
                            )
                            nc.vector.memset(vp[:, DK : DK + 1], 1.0)
                            vp_tiles.append(vp)

                        if loc:
                            # V^T, rv = mlp_w[h].V^T; perq = rv + ru + cu
                            ps_vt = ps_big.tile([P, N], F32, tag="big")
                            for kc in range(NK):
                                nc.tensor.transpose(
                                    ps_vt[0:DK, kc * P : (kc + 1) * P],
                                    vp_tiles[kc][:, 0:DK],
                                    ident,
                                )
                            vt = hsing.tile([DK, N], F32, tag="vt")
                            nc.vector.tensor_copy(vt, ps_vt[0:DK, :])
                            ps_rv = ps_big.tile([1, N], F32, tag="big")
                            for (s0, sw) in SPL:
                                nc.tensor.matmul(
                                    ps_rv[:, s0 : s0 + sw],
                                    mw_cols[hl],
                                    vt[:, s0 : s0 + sw],
                                    start=True,
                                    stop=True,
                                )
                            rv = hsing.tile([1, N], F32, tag="rv")
                            nc.vector.tensor_copy(rv, ps_rv)
                            # ru in psum: sum_dc WU[dc]^T . ut[dc]
                            ps_ru = ps_big.tile([1, N], F32, tag="big")
                            for dc in range(D // P):
                                for (s0, sw) in SPL:
                                    nc.tensor.matmul(
                                        ps_ru[:, s0 : s0 + sw],
                                        wu_chunks[dc][:, hl : hl + 1],
                                        ut_tiles[dc][:, s0 : s0 + sw],
                                        start=(dc == 0),
                                        stop=(dc == D // P - 1),
                                    )
                            perq = hsing.tile([1, N], F32, tag="perq")
                            nc.vector.tensor_scalar(
                                out=perq,
                                in0=ps_ru,
                                scalar1=cu_sb[0:1, hl : hl + 1],
                                scalar2=None,
                                op0=ALU.add,
                            )
                            nc.vector.tensor_tensor(
                                out=perq, in0=perq, in1=rv, op=ALU.add
                            )
                            nc.sync.dma_start(out=qp[64:65, :], in_=perq)
                            nc.sync.dma_start(out=qp[65:66, :], in_=ones_row)
                            nc.vector.memset(kp[64:65, :], 1.0)
                            nc.sync.dma_start(out=kp[65:66, :], in_=rv)

                        # ---- main k-block loop
                        ps_ot = ps_acc.tile([DK + 1, N], F32, tag="ot")
                        pp_tiles = []
                        for j in range(NK):
                            pp = ppool.tile([P, N], F32, tag="pp")
                            for (s0, sw) in SPL:
                                sl = slice(s0, s0 + sw)
                                ps = ps_s.tile([P, 512], F32, tag="s", name="s_ps")
                                nc.tensor.matmul(
                                    ps[:, 0:sw],
                                    kp[0:kdim, j * P : (j + 1) * P],
                                    qp[0:kdim, sl],
                                    start=True,
                                    stop=False,
                                )
                                if loc:
                                    off = (NK - 1 - j) * P + s0
                                    nc.tensor.matmul(
                                        ps[:, 0:sw],
                                        jmat_b,
                                        w_tiles[hl][:, off : off + sw],
                                        start=False,
                                        stop=False,
                                    )
                                nc.tensor.matmul(
                                    ps[:, 0:sw],
                                    ident_b,
                                    maskT[j][:, sl],
                                    start=False,
                                    stop=True,
                                )
                                nc.scalar.activation(
                                    out=pp[:, sl], in_=ps[:, 0:sw], func=AF.Exp
                                )
                                nc.tensor.matmul(
                                    ps_ot[:, sl],
                                    vp_tiles[j],
                                    pp[:, sl],
                                    start=(j == 0),
                                    stop=(j == NK - 1),
                                )
                            pp_tiles.append(pp)

                        # ---- outT -> SBUF; transpose incl. Z row; 1/Z
                        ot_sb = smallp.tile([DK + 1, N], F32, tag="otsb")
                        nc.vector.tensor_copy(ot_sb, ps_ot)
                        ps_o = ps_big.tile([P, NG, 512], F32, tag="big")
                        for qc in range(NQ):
                            g, i = divmod(qc, 4)
                            nc.tensor.transpose(
                                ps_o[:, g, i * 65 : i * 65 + 65],
                                ot_sb[0 : DK + 1, qc * P : (qc + 1) * P],
                                ident[0 : DK + 1, 0 : DK + 1],
                            )
                        rz = smallp.tile([P, NQ], F32, tag="rz")
                        for g in range(NG):
                            ng = min(4, NQ - g * 4)
                            nc.vector.reciprocal(
                                out=rz[:, g * 4 : g * 4 + ng],
                                in_=ps_o[:, g, 64 : ng * 65 : 65],
                            )
                        o_sb = smallp.tile([P, NQ * DK], F32, tag="osb")
                        for qc in range(NQ):
                            g, i = divmod(qc, 4)
                            nc.vector.tensor_scalar(
                                out=o_sb[:, qc * DK : (qc + 1) * DK],
                                in0=ps_o[:, g, i * 65 : i * 65 + DK],
                                scalar1=rz[:, qc : qc + 1],
                                scalar2=None,
                                op0=ALU.mult,
                            )
                        nc.sync.dma_start(
                            out=out_d.ap()[b, h].rearrange("(qc p) d -> p qc d", p=P),
                            in_=o_sb.rearrange("p (qc d) -> p qc d", d=DK),
                        )

                        # ---- P store: transpose chunks, scale by 1/Z
                        p_out = pl_d.ap()[b, hl] if loc else pg_d.ap()[b, h]
                        for qc in range(NQ):
                            ps_pt = ps_big.tile([P, N], F32, tag="big")
                            for kc in range(NK):
                                nc.tensor.transpose(
                                    ps_pt[:, kc * P : (kc + 1) * P],
                                    pp_tiles[kc][:, qc * P : (qc + 1) * P],
                                    ident,
                                )
                            pst = pstorep.tile([P, N], F32, tag="pst")
                            if qc % 4 == 3:
                                nc.scalar.activation(
                                    out=pst,
                                    in_=ps_pt,
                                    func=AF.Copy,
                                    bias=0.0,
                                    scale=rz[:, qc : qc + 1],
                                )
                            else:
                                nc.vector.tensor_scalar(
                                    out=pst,
                                    in0=ps_pt,
                                    scalar1=rz[:, qc : qc + 1],
                                    scalar2=None,
                                    op0=ALU.mult,
                                )
                            nc.sync.dma_start(
                                out=p_out[qc * P : (qc + 1) * P, :], in_=pst
                            )

    nc.compile()
    return nc


_NC_CACHE = None


def _get_nc():
    global _NC_CACHE
    if _NC_CACHE is None:
        _NC_CACHE = build_nc()
    return _NC_CACHE


def _shard_inputs(inputs):
    q = np.ascontiguousarray(np.asarray(inputs["query"], dtype=np.float32))
    k = np.ascontiguousarray(np.asarray(inputs["key"], dtype=np.float32))
    v = np.ascontiguousarray(np.asarray(inputs["value"], dtype=np.float32))
    mask = np.ascontiguousarray(np.asarray(inputs["mask"], dtype=np.int32))
    users = np.ascontiguousarray(np.asarray(inputs["users"], dtype=np.float32))
    rpt = np.ascontiguousarray(np.asarray(inputs["rel_pos_table"], dtype=np.float32))
    upw = np.ascontiguousarray(np.asarray(inputs["user_proj_w"], dtype=np.float32))
    upb = np.ascontiguousarray(np.asarray(inputs["user_proj_b"], dtype=np.float32))
    mlw = np.ascontiguousarray(np.asarray(inputs["mlp_w"], dtype=np.float32))
    mlb = np.ascontiguousarray(np.asarray(inputs["mlp_b"], dtype=np.float32))
    in_maps = []
    for i in range(N_CORES):
        sl = slice(i * B_LOC, (i + 1) * B_LOC)
        in_maps.append(
            {
                "q": q[sl], "k": k[sl], "v": v[sl], "mask": mask[sl],
                "users": users[sl], "rpt": rpt, "upw": upw, "upb": upb,
                "mlw": mlw, "mlb": mlb,
            }
        )
    return in_maps


def run_sharded(inputs, trace=False):
    nc = _get_nc()
    in_maps = _shard_inputs(inputs)
    res = run_bass_kernel_spmd(
        nc, in_maps, core_ids=list(range(N_CORES)), trace=trace
    )
    outs = res.results
    out = np.concatenate([r["out"] for r in outs], axis=0)
    pg = np.concatenate([r["pg"] for r in outs], axis=0)
    pl = np.concatenate([r["pl"] for r in outs], axis=0)
    return (out, pg, pl), res


def kernel(**inputs):
    (out, pg, pl), _ = run_sharded(inputs, trace=False)
    return (out, pg, pl)


if __name__ == "__main__":
    build_nc()
    print("build ok")


# revision 17
# speedup vs baseline: 1.4090x; 1.1400x over previous
"""AdaptAttention Trainium2 kernel.

8 NeuronCores, data-parallel over batch (16 batches -> 2 per core).
Per (batch, head): computes S'[k,q] (transposed-domain scores) fully on
TensorE PSUM accumulation, exp on ScalarE, P@V + row-sums via a
ones-column, P^T store tiles via PE transpose with 1/Z fused into the
PSUM->SBUF copy.

Returns (out, p_attn_g, p_attn_l) matching the reference module.
"""

import math
import sys

import numpy as np

sys.path.insert(0, "/opt/trn_rl_repo")

import concourse.bacc as bacc
import concourse.bass as bass
import concourse.mybir as mybir
from concourse.bass_utils import run_bass_kernel_spmd
from concourse.masks import make_identity
from concourse.tile import TileContext

AF = mybir.ActivationFunctionType
ALU = mybir.AluOpType
F32 = mybir.dt.float32
BF16 = mybir.dt.bfloat16
I32 = mybir.dt.int32

N = 1024
DK = 64
H = 8
H_GLB = 4
H_LOC = 4
D = 512
B_LOC = 2  # batches per core
N_CORES = 8
P = 128  # partitions
MASKV = 1.0e12


def _derived():
    global NK, NQ, NC2, WN, SPL, NG
    NK = N // P  # k-blocks
    NQ = N // P  # q-chunks
    NC2 = 2 * N // P  # chunks of the rel-pos fold
    WN = N + (NK - 1) * P  # window width
    # <=512-wide column splits of N (psum bank per matmul)
    SPL = [(i, min(512, N - i)) for i in range(0, N, 512)]
    NG = (NQ + 3) // 4  # groups of 4 q-chunks for the out-transpose banks


_derived()


def _affine_fill_diag(nc, ap, val, base, cm, step):
    """Fill ap where (base + cm*partition + step*col) == 0 with val."""
    nc.gpsimd.affine_select(
        out=ap,
        in_=ap,
        compare_op=ALU.not_equal,
        fill=val,
        base=base,
        pattern=[[step, ap.shape[1]]],
        channel_multiplier=cm,
    )


def _load(nc, pool, dram_ap, shape, dtype, tag):
    t = pool.tile(shape, dtype, tag=tag)
    nc.sync.dma_start(out=t, in_=dram_ap)
    return t


def build_nc():
    nc = bacc.Bacc(
        "TRN2",
        target_bir_lowering=False,
        debug=False,
        enable_asserts=False,
        num_devices=N_CORES,
    )

    # ---- I/O ----
    q_d = nc.dram_tensor("q", [B_LOC, H, N, DK], F32, kind="ExternalInput")
    k_d = nc.dram_tensor("k", [B_LOC, H, N, DK], F32, kind="ExternalInput")
    v_d = nc.dram_tensor("v", [B_LOC, H, N, DK], F32, kind="ExternalInput")
    mask_d = nc.dram_tensor("mask", [B_LOC, 1, N, N], I32, kind="ExternalInput")
    users_d = nc.dram_tensor("users", [B_LOC, N, D], F32, kind="ExternalInput")
    rpt_d = nc.dram_tensor("rpt", [2 * N - 1, H_LOC * DK], F32, kind="ExternalInput")
    upw_d = nc.dram_tensor("upw", [D, H_LOC * DK], F32, kind="ExternalInput")
    upb_d = nc.dram_tensor("upb", [H_LOC * DK], F32, kind="ExternalInput")
    mlw_d = nc.dram_tensor("mlw", [H_LOC, DK], F32, kind="ExternalInput")
    mlb_d = nc.dram_tensor("mlb", [H_LOC], F32, kind="ExternalInput")

    out_d = nc.dram_tensor("out", [B_LOC, H, N, DK], F32, kind="ExternalOutput")
    pg_d = nc.dram_tensor("pg", [B_LOC, H_GLB, N, N], F32, kind="ExternalOutput")
    pl_d = nc.dram_tensor("pl", [B_LOC, H_LOC, N, N], F32, kind="ExternalOutput")

    scale = 1.0 / math.sqrt(DK)

    with TileContext(nc) as tc:
        with (
            tc.tile_pool(name="consts", bufs=1) as consts,
            tc.tile_pool(name="wwin", bufs=1) as wwin,
            tc.tile_pool(name="dram", bufs=1, space="DRAM") as dpool,
            tc.tile_pool(name="ps_s", bufs=2, space="PSUM") as ps_s,
            tc.tile_pool(name="ps_acc", bufs=1, space="PSUM") as ps_acc,
            tc.tile_pool(name="ps_big", bufs=2, space="PSUM") as ps_big,
        ):
            # ================= prologue (once) =================
            ident = consts.tile([P, P], F32)
            make_identity(nc, ident)
            jmat_f = consts.tile([P, P], F32)
            nc.gpsimd.memset(jmat_f, 0.0)
            _affine_fill_diag(nc, jmat_f, 1.0, base=-(P - 1), cm=1, step=1)
            ident_b = consts.tile([P, P], BF16)
            nc.vector.tensor_copy(ident_b, ident)
            jmat_b = consts.tile([P, P], BF16)
            nc.vector.tensor_copy(jmat_b, jmat_f)

            # mlp_w broadcast [128, 256], flat row [1, 256], per-head cols
            mw_b = consts.tile([P, H_LOC * DK], F32)
            nc.sync.dma_start(
                out=mw_b,
                in_=bass.AP(tensor=mlw_d, offset=0, ap=[[0, P], [1, H_LOC * DK]]),
            )
            ones_row = consts.tile([1, N], F32)
            nc.vector.memset(ones_row, 1.0)
            mw_cols = []
            for h in range(H_LOC):
                t = consts.tile([DK, 1], F32, tag=f"mwc{h}")
                nc.sync.dma_start(
                    out=t,
                    in_=bass.AP(tensor=mlw_d, offset=h * DK, ap=[[1, DK], [0, 1]]),
                )
                mw_cols.append(t)

            # W_U [512, 4] chunks + c_u' [1, 4]; rel-pos fold -> W windows
            wu_chunks = [consts.tile([P, H_LOC], F32, tag=f"wu{dc}", name=f"wu{dc}") for dc in range(D // P)]
            cu_sb = consts.tile([1, H_LOC], F32)
            w_tiles = [wwin.tile([P, WN], BF16, tag=f"w{h}", name=f"wwin{h}") for h in range(H_LOC)]
            t_rev_dram = dpool.tile([H_LOC, 2 * N], F32)

            with tc.tile_pool(name="prol", bufs=2) as prol:
                mw_row = prol.tile([1, H_LOC * DK], F32, tag="mwrow")
                nc.sync.dma_start(
                    out=mw_row,
                    in_=bass.AP(
                        tensor=mlw_d, offset=0, ap=[[0, 1], [1, H_LOC * DK]]
                    ),
                )
                for dc in range(D // P):
                    upw_t = prol.tile([P, H_LOC * DK], F32, tag="upw")
                    nc.sync.dma_start(
                        out=upw_t, in_=upw_d.ap()[dc * P : (dc + 1) * P, :]
                    )
                    tmp = prol.tile([P, H_LOC * DK], F32, tag="wutmp")
                    nc.vector.tensor_tensor(out=tmp, in0=upw_t, in1=mw_b, op=ALU.mult)
                    nc.vector.tensor_reduce(
                        out=wu_chunks[dc],
                        in_=tmp.rearrange("p (h d) -> p h d", d=DK),
                        axis=mybir.AxisListType.X,
                        op=ALU.add,
                    )

                upb_t = prol.tile([1, H_LOC * DK], F32, tag="upb")
                nc.sync.dma_start(
                    out=upb_t,
                    in_=bass.AP(tensor=upb_d, offset=0, ap=[[0, 1], [1, H_LOC * DK]]),
                )
                cu_tmp = prol.tile([1, H_LOC * DK], F32, tag="cutmp")
                nc.vector.tensor_tensor(out=cu_tmp, in0=upb_t, in1=mw_row, op=ALU.mult)
                nc.vector.tensor_reduce(
                    out=cu_sb,
                    in_=cu_tmp.rearrange("p (h d) -> p h d", d=DK),
                    axis=mybir.AxisListType.X,
                    op=ALU.add,
                )
                mlb_t = prol.tile([1, H_LOC], F32, tag="mlb")
                nc.sync.dma_start(
                    out=mlb_t,
                    in_=bass.AP(tensor=mlb_d, offset=0, ap=[[0, 1], [1, H_LOC]]),
                )
                nc.vector.tensor_tensor(out=cu_sb, in0=cu_sb, in1=mlb_t, op=ALU.add)

                # T4[p, c, h] = t~[128c + p], t~[i] = t[i-1] (t~[0] junk)
                t4 = prol.tile([P, NC2, H_LOC], F32, tag="t4")
                for c in range(NC2):
                    rc = prol.tile([P, H_LOC * DK], F32, tag="rc")
                    lo = c * P - 1
                    if lo < 0:
                        nc.sync.dma_start(out=rc[1:P, :], in_=rpt_d.ap()[0 : P - 1, :])
                        nc.vector.memset(rc[0:1, :], 0.0)
                    else:
                        nc.sync.dma_start(out=rc, in_=rpt_d.ap()[lo : lo + P, :])
                    tmp2 = prol.tile([P, H_LOC * DK], F32, tag="rcm")
                    nc.vector.tensor_tensor(out=tmp2, in0=rc, in1=mw_b, op=ALU.mult)
                    nc.vector.tensor_reduce(
                        out=t4[:, c],
                        in_=tmp2.rearrange("p (h d) -> p h d", d=DK),
                        axis=mybir.AxisListType.X,
                        op=ALU.add,
                    )

                # t_rev[h, i] = t~[2047 - i]: J-flip partitions, reverse cols
                for h in range(H_LOC):
                    ps_jt = ps_big.tile([P, NC2], F32, tag="big")
                    t4h = prol.tile([P, NC2], BF16, tag="t4h")
                    nc.vector.tensor_copy(out=t4h, in_=t4[:, :, h])
                    nc.tensor.matmul(ps_jt, jmat_b, t4h, start=True, stop=True)
                    tr_sb = prol.tile([P, NC2], F32, tag="trsb")
                    for c in range(NC2):
                        nc.vector.tensor_copy(
                            out=tr_sb[:, NC2 - 1 - c : NC2 - c], in_=ps_jt[:, c : c + 1]
                        )
                    nc.sync.dma_start(
                        out=bass.AP(
                            tensor=t_rev_dram.tensor,
                            offset=t_rev_dram.offset + h * 2 * N,
                            ap=[[1, P], [P, NC2]],
                        ),
                        in_=tr_sb,
                    )
                    # W_h[p, c] = t_rev[p + c] (overlapping windows), bf16
                    wtmp = prol.tile([P, WN], F32, tag="wtmp")
                    nc.sync.dma_start(
                        out=wtmp,
                        in_=bass.AP(
                            tensor=t_rev_dram.tensor,
                            offset=t_rev_dram.offset + h * 2 * N,
                            ap=[[1, P], [1, WN]],
                        ),
                    )
                    nc.vector.tensor_copy(w_tiles[h], wtmp)

            # ================= main =================
            with (
                tc.tile_pool(name="maskT", bufs=1) as maskp,
                tc.tile_pool(name="mload", bufs=2) as mload,
                tc.tile_pool(name="batch", bufs=1) as batchp,
                tc.tile_pool(name="qk", bufs=2) as qkp,
                tc.tile_pool(name="vpool", bufs=2) as vpoolp,
                tc.tile_pool(name="hsing", bufs=1) as hsing,
                tc.tile_pool(name="pp", bufs=18) as ppool,
                tc.tile_pool(name="pstore", bufs=2) as pstorep,
                tc.tile_pool(name="small", bufs=2) as smallp,
            ):
                for b in range(B_LOC):
                    # ---- maskT tiles [128k, 1024q] bf16 (0 valid / -1e12)
                    maskT = [maskp.tile([P, N], BF16, tag=f"mt{kc}", name=f"mt{kc}") for kc in range(NK)]
                    for qc in range(NQ):
                        mi = mload.tile([P, N], I32, tag="mi")
                        nc.sync.dma_start(
                            out=mi, in_=mask_d.ap()[b, 0, qc * P : (qc + 1) * P, :]
                        )
                        mf = mload.tile([P, N], F32, tag="mf")
                        nc.scalar.activation(
                            out=mf, in_=mi, func=AF.Copy, bias=-MASKV, scale=MASKV
                        )
                        ps_mt = ps_big.tile([P, N], F32, tag="big")
                        for kc in range(NK):
                            nc.tensor.transpose(
                                ps_mt[:, kc * P : (kc + 1) * P],
                                mf[:, kc * P : (kc + 1) * P],
                                ident,
                            )
                        for kc in range(NK):
                            nc.vector.tensor_copy(
                                maskT[kc][:, qc * P : (qc + 1) * P],
                                ps_mt[:, kc * P : (kc + 1) * P],
                            )

                    # ---- users^T (persists for this batch, feeds per-head ru)
                    ut_tiles = []
                    for dc in range(D // P):
                        ps_ut = ps_big.tile([P, N], F32, tag="big")
                        for qc in range(NQ):
                            u_in = _load(
                                nc,
                                batchp,
                                users_d.ap()[b][
                                    qc * P : (qc + 1) * P, dc * P : (dc + 1) * P
                                ],
                                [P, P],
                                F32,
                                tag="uld",
                            )
                            nc.tensor.transpose(
                                ps_ut[:, qc * P : (qc + 1) * P], u_in, ident
                            )
                        ut = batchp.tile([P, N], F32, tag=f"ut{dc}")
                        nc.vector.tensor_copy(ut, ps_ut)
                        ut_tiles.append(ut)

                    # ================= per head =================
                    for h in [0, 1, 2, 3, 4, 5, 6, 7][:H_GLB] + list(range(H_GLB, H)):
                        loc = h >= H_GLB
                        hl = h - H_GLB
                        kdim = 66 if loc else 64

                        # Q^T (scaled) -> qp rows 0..63
                        qp = qkp.tile([66, N], F32, tag="qp")
                        qld = qkp.tile([P, NQ, DK], F32, tag="qld")
                        nc.sync.dma_start(
                            out=qld,
                            in_=q_d.ap()[b, h].rearrange("(c p) d -> p c d", p=P),
                        )
                        ps_qt = ps_big.tile([P, N], F32, tag="big")
                        for qc in range(NQ):
                            nc.tensor.transpose(
                                ps_qt[0:DK, qc * P : (qc + 1) * P], qld[:, qc], ident
                            )
                        nc.vector.tensor_scalar_mul(qp[0:DK, :], ps_qt[0:DK, :], scale)

                        # K^T -> kp rows 0..63
                        kp = qkp.tile([66, N], F32, tag="kp")
                        kld = qkp.tile([P, NK, DK], F32, tag="kld")
                        nc.sync.dma_start(
                            out=kld,
                            in_=k_d.ap()[b, h].rearrange("(c p) d -> p c d", p=P),
                        )
                        ps_kt = ps_big.tile([P, N], F32, tag="big")
                        for kc in range(NK):
                            nc.tensor.transpose(
                                ps_kt[0:DK, kc * P : (kc + 1) * P], kld[:, kc], ident
                            )
                        nc.vector.tensor_copy(kp[0:DK, :], ps_kt[0:DK, :])

                        # V (bf16, ones column appended): one cast DMA + memset
                        vbig = vpoolp.tile([P, NK, DK + 1], BF16, tag="vbig")
                        nc.gpsimd.dma_start(
                            out=vbig[:, :, 0:DK],
                            in_=v_d.ap()[b, h].rearrange("(c p) d -> p c d", p=P),
                        )
                        nc.vector.memset(vbig[:, :, DK : DK + 1], 1.0)
                        vp_tiles = [vbig[:, kc] for kc in range(NK)]

                        if loc:
                            # V^T, rv = mlp_w[h].V^T; perq = rv + ru + cu
                            ps_vt = ps_big.tile([P, N], BF16, tag="big", name="ps_vt")
                            for kc in range(NK):
                                nc.tensor.transpose(
                                    ps_vt[0:DK, kc * P : (kc + 1) * P],
                                    vp_tiles[kc][:, 0:DK],
                                    ident_b,
                                )
                            vt = hsing.tile([DK, N], F32, tag="vt")
                            nc.vector.tensor_copy(vt, ps_vt[0:DK, :])
                            ps_rv = ps_big.tile([1, N], F32, tag="big")
                            for (s0, sw) in SPL:
                                nc.tensor.matmul(
                                    ps_rv[:, s0 : s0 + sw],
                                    mw_cols[hl],
                                    vt[:, s0 : s0 + sw],
                                    start=True,
                                    stop=True,
                                )
                            rv = hsing.tile([1, N], F32, tag="rv")
                            nc.vector.tensor_copy(rv, ps_rv)
                            # ru in psum: sum_dc WU[dc]^T . ut[dc]
                            ps_ru = ps_big.tile([1, N], F32, tag="big")
                            for dc in range(D // P):
                                for (s0, sw) in SPL:
                                    nc.tensor.matmul(
                                        ps_ru[:, s0 : s0 + sw],
                                        wu_chunks[dc][:, hl : hl + 1],
                                        ut_tiles[dc][:, s0 : s0 + sw],
                                        start=(dc == 0),
                                        stop=(dc == D // P - 1),
                                    )
                            perq = hsing.tile([1, N], F32, tag="perq")
                            nc.vector.tensor_scalar(
                                out=perq,
                                in0=ps_ru,
                                scalar1=cu_sb[0:1, hl : hl + 1],
                                scalar2=None,
                                op0=ALU.add,
                            )
                            nc.vector.tensor_tensor(
                                out=perq, in0=perq, in1=rv, op=ALU.add
                            )
                            nc.sync.dma_start(out=qp[64:65, :], in_=perq)
                            nc.sync.dma_start(out=qp[65:66, :], in_=ones_row)
                            nc.vector.memset(kp[64:65, :], 1.0)
                            nc.sync.dma_start(out=kp[65:66, :], in_=rv)

                        # ---- main k-block loop
                        ps_ot = ps_acc.tile([DK + 1, N], F32, tag="ot")
                        pp_tiles = []
                        for j in range(NK):
                            pp = ppool.tile([P, N], BF16, tag="pp")
                            for (s0, sw) in SPL:
                                sl = slice(s0, s0 + sw)
                                ps = ps_s.tile([P, 512], F32, tag="s", name="s_ps")
                                nc.tensor.matmul(
                                    ps[:, 0:sw],
                                    kp[0:kdim, j * P : (j + 1) * P],
                                    qp[0:kdim, sl],
                                    start=True,
                                    stop=False,
                                )
                                if loc:
                                    off = (NK - 1 - j) * P + s0
                                    nc.tensor.matmul(
                                        ps[:, 0:sw],
                                        jmat_b,
                                        w_tiles[hl][:, off : off + sw],
                                        start=False,
                                        stop=False,
                                    )
                                nc.tensor.matmul(
                                    ps[:, 0:sw],
                                    ident_b,
                                    maskT[j][:, sl],
                                    start=False,
                                    stop=True,
                                )
                                nc.scalar.activation(
                                    out=pp[:, sl], in_=ps[:, 0:sw], func=AF.Exp
                                )
                                nc.tensor.matmul(
                                    ps_ot[:, sl],
                                    vp_tiles[j],
                                    pp[:, sl],
                                    start=(j == 0),
                                    stop=(j == NK - 1),
                                )
                            pp_tiles.append(pp)

                        # ---- outT -> SBUF; transpose incl. Z row; 1/Z
                        ot_sb = smallp.tile([DK + 1, N], F32, tag="otsb")
                        nc.vector.tensor_copy(ot_sb, ps_ot)
                        ps_o = ps_big.tile([P, NG, 512], F32, tag="big")
                        for qc in range(NQ):
                            g, i = divmod(qc, 4)
                            nc.tensor.transpose(
                                ps_o[:, g, i * 65 : i * 65 + 65],
                                ot_sb[0 : DK + 1, qc * P : (qc + 1) * P],
                                ident[0 : DK + 1, 0 : DK + 1],
                            )
                        rz = smallp.tile([P, NQ], F32, tag="rz")
                        for g in range(NG):
                            ng = min(4, NQ - g * 4)
                            nc.vector.reciprocal(
                                out=rz[:, g * 4 : g * 4 + ng],
                                in_=ps_o[:, g, 64 : ng * 65 : 65],
                            )
                        o_sb = smallp.tile([P, NQ * DK], F32, tag="osb")
                        for g in range(NG):
                            ng = min(4, NQ - g * 4)
                            in0 = bass.AP(
                                tensor=ps_o.tensor,
                                offset=ps_o.offset + g * 512,
                                ap=[ps_o.ap[0], [65, ng], [1, DK]],
                            )
                            in1 = bass.AP(
                                tensor=rz.tensor,
                                offset=rz.offset + g * 4,
                                ap=[rz.ap[0], [1, ng], [0, DK]],
                            )
                            nc.vector.tensor_tensor(
                                out=o_sb[:, g * 4 * DK : (g * 4 + ng) * DK].rearrange(
                                    "p (c d) -> p c d", d=DK
                                ),
                                in0=in0,
                                in1=in1,
                                op=ALU.mult,
                            )
                        nc.sync.dma_start(
                            out=out_d.ap()[b, h].rearrange("(qc p) d -> p qc d", p=P),
                            in_=o_sb.rearrange("p (qc d) -> p qc d", d=DK),
                        )

                        # ---- P store: transpose chunks, scale by 1/Z
                        p_out = pl_d.ap()[b, hl] if loc else pg_d.ap()[b, h]
                        for qc in range(NQ):
                            ps_pt = ps_big.tile([P, N], BF16, tag="big", name="ps_pt")
                            for kc in range(NK):
                                nc.tensor.transpose(
                                    ps_pt[:, kc * P : (kc + 1) * P],
                                    pp_tiles[kc][:, qc * P : (qc + 1) * P],
                                    ident_b,
                                )
                            pst = pstorep.tile([P, N], F32, tag="pst")
                            if qc % 2 == 1:
                                nc.scalar.activation(
                                    out=pst,
                                    in_=ps_pt,
                                    func=AF.Copy,
                                    bias=0.0,
                                    scale=rz[:, qc : qc + 1],
                                )
                            else:
                                nc.vector.tensor_scalar(
                                    out=pst,
                                    in0=ps_pt,
                                    scalar1=rz[:, qc : qc + 1],
                                    scalar2=None,
                                    op0=ALU.mult,
                                )
                            nc.sync.dma_start(
                                out=p_out[qc * P : (qc + 1) * P, :], in_=pst
                            )

    nc.compile()
    return nc


_NC_CACHE = None


def _get_nc():
    global _NC_CACHE
    if _NC_CACHE is None:
        _NC_CACHE = build_nc()
    return _NC_CACHE


def _shard_inputs(inputs):
    q = np.ascontiguousarray(np.asarray(inputs["query"], dtype=np.float32))
    k = np.ascontiguousarray(np.asarray(inputs["key"], dtype=np.float32))
    v = np.ascontiguousarray(np.asarray(inputs["value"], dtype=np.float32))
    mask = np.ascontiguousarray(np.asarray(inputs["mask"], dtype=np.int32))
    users = np.ascontiguousarray(np.asarray(inputs["users"], dtype=np.float32))
    rpt = np.ascontiguousarray(np.asarray(inputs["rel_pos_table"], dtype=np.float32))
    upw = np.ascontiguousarray(np.asarray(inputs["user_proj_w"], dtype=np.float32))
    upb = np.ascontiguousarray(np.asarray(inputs["user_proj_b"], dtype=np.float32))
    mlw = np.ascontiguousarray(np.asarray(inputs["mlp_w"], dtype=np.float32))
    mlb = np.ascontiguousarray(np.asarray(inputs["mlp_b"], dtype=np.float32))
    in_maps = []
    for i in range(N_CORES):
        sl = slice(i * B_LOC, (i + 1) * B_LOC)
        in_maps.append(
            {
                "q": q[sl], "k": k[sl], "v": v[sl], "mask": mask[sl],
                "users": users[sl], "rpt": rpt, "upw": upw, "upb": upb,
                "mlw": mlw, "mlb": mlb,
            }
        )
    return in_maps


def run_sharded(inputs, trace=False):
    nc = _get_nc()
    in_maps = _shard_inputs(inputs)
    res = run_bass_kernel_spmd(
        nc, in_maps, core_ids=list(range(N_CORES)), trace=trace
    )
    outs = res.results
    out = np.concatenate([r["out"] for r in outs], axis=0)
    pg = np.concatenate([r["pg"] for r in outs], axis=0)
    pl = np.concatenate([r["pl"] for r in outs], axis=0)
    return (out, pg, pl), res


def kernel(**inputs):
    (out, pg, pl), _ = run_sharded(inputs, trace=False)
    return (out, pg, pl)


if __name__ == "__main__":
    build_nc()
    print("build ok")
